# revision 70
# baseline (speedup 1.0000x reference)
"""GPT-2-style 6-layer transformer forward on 8 trn2 NeuronCores — v3.

Sharding: 2 groups of 4 cores (one group per batch element). Within a group,
the 8 token-blocks (128 tokens each) are assigned block-cyclically: core j of
the group owns blocks {j, 7-j}. The residual stream lives TRANSPOSED as
[C(partitions), 256 tokens] per core.

v3 highlights vs v2:
 - int8 KV AllGather wire (halves collective bytes): V rides alpha folded
   into the V weights with the softmax-denominator ones-column set to alpha
   (ratio cancels exactly); K rides beta folded into K weights with 1/beta
   folded into Q weights (scores unchanged). Round-to-nearest via the fp32
   magic-number trick + explicit clip, so the int8 conversion is exact under
   any dtype-cast rounding mode. Dequant happens in the fetch DMA (SWDGE
   int8->bf16 cast); all on-chip matmuls stay bf16.
 - Token-sharded head: each core computes its own 256 tokens x full 32000
   vocab from the LOCAL final-LN output — the final x AllGather is gone.
   Head weights stream as 32x 1.5MB chunks, hidden under the head matmuls.
 - Softmax exp batched into one [128,1536] PSUM strip per (head-pair,
   row-half): one Exp per strip ((N+352)/1.2 Act cost amortized), masks as
   2 batched DVE ops. PSUM strips use one accumulation GROUP per 2KB bank
   (start on first MM, stop on last; a second start would pending-zero the
   whole bank).
 - ff2 runs k-outer into a single 3-bank strip (one group per bank), with
   the 24 k-tiles streamed as 4x 1.15MB DMAs — frees ~20KB SBUF for the
   head stream.
 - LN: bf16 [x|x2] shadow maintained at residual writes; stats matmuls
   fused into the residual loops; rstd = exp(-0.5*ln(var+eps)) so LN shares
   the natural_log_exp Act table with the softmax (the single-function
   ln/exp table sets are disabled via _patch_act_tables to avoid 2 extra
   table loads per layernorm).
 - All per-layer weight DMA issues hoisted to the layer top (SP queue
   order), K/V staged on the scalar HWDGE queue, KV fetch on gpsimd; K
   regions fetched before V (scores need only K).

kernel(**inputs) -> np.ndarray [2, 1024, 32000] float32.
"""

import sys

for _p in ("/opt/trn_rl_repo", "/opt/pypackages"):
    if _p not in sys.path:
        sys.path.append(_p)

import numpy as np
import ml_dtypes

import concourse.bass as bass
import concourse.mybir as mybir
import concourse.tile as tile
from concourse import bacc
from concourse.bass_utils import run_bass_kernel_spmd

F32 = mybir.dt.float32
BF16 = mybir.dt.bfloat16
I8 = mybir.dt.int8

# int8 wire scales for the KV AllGather payload. K rides beta (folded into
# the K weights, inverse folded into Q weights: scores unchanged). V rides
# alpha (folded into V weights/bias; the denominator ones-column is alpha
# too, so the softmax ratio cancels it exactly).
ALPHA = 48.0
BETA = 48.0
MAGIC = 12582912.0  # 2^23 + 2^22: (x + MAGIC) - MAGIC == round-to-nearest-even
AF = mybir.ActivationFunctionType
ALU = mybir.AluOpType

# model dims
V, T, L, C, H, DFF = 32000, 1024, 6, 768, 12, 3072
DH = C // H          # 64
CT = C // 128        # 6 c-tiles
FT = DFF // 128      # 24 ff d-tiles
TOK = 256            # tokens per core (2 blocks of 128)
NB = T // 128        # 8 token blocks per group
VSH = V // 4         # 8000 vocab shard per core
EPS = 1e-5
VH = DH + 1          # 65: per-head V columns incl. ones col
VW = H * VH          # 780: V columns per 128-token block
KVW = CT * TOK + 2 * VW  # 3096: AllGather payload columns per core

HV_CHUNK = 500
HV2 = 1000           # head vocab chunk (token-sharded head, full vocab per core)
NH2 = V // HV2       # 32 chunks

REPLICA_GROUPS = [[0, 1, 2, 3], [4, 5, 6, 7]]


_GAT_PATCHED = False


def _patch_act_tables():
    """Steer the act-table-load pass away from the single-function sets so
    Ln and Exp both resolve to natural_log_exp_and_others (one load instead
    of two per layernorm; the softmax exps share it too). Set ids/positions
    are preserved — only eligibility changes — so the emitted
    act_func_set_id values stay valid for the real compiler."""
    global _GAT_PATCHED
    if _GAT_PATCHED:
        return
    import functools
    import concourse.bacc as _bacc_mod
    from concourse.hw_specs import get_activation_tables as _orig_gat

    @functools.cache
    def _patched(arch):
        t = dict(_orig_gat(arch))
        for kill in ("natural_log", "exp_and_others"):
            if kill in t:
                t[kill] = set()
        return t

    _bacc_mod.get_activation_tables = _patched
    _GAT_PATCHED = True


def build_kernel(n_layers=L, repeats=1, mock_cc=False):
    _patch_act_tables()
    nc = bacc.Bacc("TRN2", target_bir_lowering=False, debug=False,
                   num_devices=1 if mock_cc else 8)

    # ---- dram parameters (per-core inputs, host pre-arranged) ----
    x0_d = nc.declare_dram_parameter("x0", [128, CT * TOK], F32, isOutput=False)
    wqk_d = nc.declare_dram_parameter("wqk", [L, 128, CT * 2 * C], BF16, isOutput=False)
    wv_d = nc.declare_dram_parameter("wv", [L, 128, CT * C], BF16, isOutput=False)
    wproj_d = nc.declare_dram_parameter("wproj", [L, 128, CT * C], BF16, isOutput=False)
    wff1_d = nc.declare_dram_parameter("wff1", [L, 128, CT * DFF], BF16, isOutput=False)
    wff2_d = nc.declare_dram_parameter("wff2", [L, 128, FT * C], BF16, isOutput=False)
    whead_d = nc.declare_dram_parameter("whead", [NH2, 128, CT * HV2], BF16, isOutput=False)
    bqk_d = nc.declare_dram_parameter("bqk", [L, 128, 2 * CT], F32, isOutput=False)
    vbias_d = nc.declare_dram_parameter("vbias", [L, 128, C], BF16, isOutput=False)
    bproj_d = nc.declare_dram_parameter("bproj", [L, 128, CT], F32, isOutput=False)
    bff1_d = nc.declare_dram_parameter("bff1", [L, 128, FT], F32, isOutput=False)
    bff2_d = nc.declare_dram_parameter("bff2", [L, 128, CT], F32, isOutput=False)
    lnp_d = nc.declare_dram_parameter("lnp", [L, 128, 4 * CT], F32, isOutput=False)
    lnf_d = nc.declare_dram_parameter("lnf", [128, 2 * CT], F32, isOutput=False)
    maskA_d = nc.declare_dram_parameter("maskA", [128, 4 * 128], BF16, isOutput=False)
    maskB_d = nc.declare_dram_parameter("maskB", [128, 4 * 128], BF16, isOutput=False)
    ones_d = nc.declare_dram_parameter("ones", [128, 128], BF16, isOutput=False)
    # token-sharded head output: own 2 blocks x full vocab
    logits_d = nc.declare_dram_parameter("logits", [2 * 128, V], BF16, isOutput=True)

    from contextlib import ExitStack

    with tile.TileContext(nc) as tc:
        with ExitStack() as _stk:
            _p = lambda *a, **k: _stk.enter_context(tc.tile_pool(*a, **k))  # noqa: E731
            constp = _p(name="const", bufs=1)
            xp = _p(name="x", bufs=1)
            actp = _p(name="act", bufs=1)
            pp = _p(name="pstrip", bufs=3)
            statsp = _p(name="stats", bufs=4)
            kvsbp = _p(name="kvsb", bufs=4)
            wqkp = _p(name="wqk", bufs=6)
            wvp = _p(name="wv", bufs=6)
            wprojp = _p(name="wproj", bufs=6)
            wff1p = _p(name="wff1", bufs=6)
            wff2p = _p(name="wff2", bufs=2)
            biasp = _p(name="bias", bufs=2)
            pmm = _p(name="pmm", bufs=2, space="PSUM")
            pscore = _p(name="pscore", bufs=2, space="PSUM")
            dramp = _p(name="dram", bufs=2, space="DRAM")
            # constants
            ones_t = constp.tile([128, 128], BF16, tag="ones")
            nc.sync.dma_start(ones_t[:], ones_d[:])
            maskA_t = constp.tile([128, 4 * 128], BF16, tag="maskA")
            nc.sync.dma_start(maskA_t[:], maskA_d[:])
            maskB_t = constp.tile([128, 4 * 128], BF16, tag="maskB")
            nc.sync.dma_start(maskB_t[:], maskB_d[:])
            lnf_t = constp.tile([128, 2 * CT], F32, tag="lnf")
            nc.sync.dma_start(lnf_t[:], lnf_d[:])
            eps_t = constp.tile([128, 1], F32, tag="eps")
            nc.gpsimd.memset(eps_t[:], EPS)

            # residual stream (fp32, transposed, persistent across layers)
            x_t = xp.tile([128, CT * TOK], F32, tag="x")
            nc.sync.dma_start(x_t[:], x0_d[:])
            # bf16 shadow: per c-tile [xbf(256) | x2(256)]
            sh = xp.tile([128, CT * 512], BF16, tag="sh")

            def shadow_update(c):
                xs = slice(c * TOK, (c + 1) * TOK)
                nc.vector.tensor_copy(sh[:, c * 512:c * 512 + 256], x_t[:, xs])
                nc.vector.tensor_tensor(sh[:, c * 512 + 256:(c + 1) * 512],
                                        sh[:, c * 512:c * 512 + 256],
                                        sh[:, c * 512:c * 512 + 256], ALU.mult)

            # persistent local-V tile; ones columns written once (= ALPHA so the
            # softmax denominator rides the same scale as the int8 V payload)
            vloc = xp.tile([128, 2 * VW], BF16, tag="vloc")
            nc.gpsimd.memset(
                vloc[:].rearrange("p (b h c) -> p b h c", h=H, c=VH)[:, :, :, DH:VH],
                ALPHA)

            def round_clip_i8(dst_i8, src_bf):
                """dst = int8(clip(round(src))), rounding src IN PLACE (src is
                dead after staging). Exact round-to-nearest via the fp32
                magic-number trick — safe under any dtype-cast rounding mode
                since the converted values are exact integers in [-127,127]."""
                nc.vector.tensor_scalar(src_bf, src_bf, MAGIC, MAGIC,
                                        ALU.add, ALU.subtract)
                nc.vector.tensor_scalar(dst_i8, src_bf, 127.0, -127.0,
                                        ALU.min, ALU.max)

            def ln_stats_tile():
                """[x-sum | x2-sum] accumulator (pscore-tag tile, cols 0:512)."""
                st_acc = pscore.tile([128, 1536], F32, tag="sc")
                return st_acc

            def ln_stats_mm(ps, c):
                nc.tensor.matmul(ps[:, 0:512], lhsT=ones_t[:],
                                 rhs=sh[:, c * 512:(c + 1) * 512],
                                 start=(c == 0), stop=(c == CT - 1),
                                 skip_group_check=True)

            def layernorm_fin(ps, gamma_ap, beta_ap, out_tag):
                """LN finalize from accumulated [x|x2] sums.
                Returns bf16 tile [128, CT*TOK]. gamma/beta: [128, CT] slices."""
                mu = statsp.tile([128, TOK], F32, tag="stats", bufs=3)
                nc.vector.tensor_scalar_mul(mu[:], ps[:, 0:256], 1.0 / C)
                # u = psxx - mu*psx = C*var; the 1/C folds into Ln's scale
                var = statsp.tile([128, TOK], F32, tag="stats", bufs=3)
                nc.vector.tensor_tensor(var[:], mu[:], ps[:, 0:256], ALU.mult)
                nc.vector.tensor_tensor(var[:], ps[:, 256:512], var[:], ALU.subtract)
                # rstd = exp(-0.5 * ln(u/C + eps)) — stays in natural_log_exp set
                lnv = statsp.tile([128, TOK], F32, tag="stats", bufs=3)
                nc.scalar.activation(lnv[:], var[:], AF.Ln, bias=eps_t[:, 0:1],
                                     scale=1.0 / C)
                rstd = statsp.tile([128, TOK], F32, tag="stats", bufs=3)
                nc.scalar.activation(rstd[:], lnv[:], AF.Exp, scale=-0.5)
                h = actp.tile([128, CT * TOK], BF16, tag=out_tag)
                tmp = statsp.tile([128, TOK], F32, tag="stats", bufs=3)
                for c in range(CT):
                    s = slice(c * TOK, (c + 1) * TOK)
                    nc.vector.tensor_tensor(tmp[:], x_t[:, s], mu[:], ALU.subtract)
                    nc.vector.tensor_tensor(tmp[:], tmp[:], rstd[:], ALU.mult)
                    nc.vector.tensor_scalar(h[:, s], tmp[:],
                                            gamma_ap[:, c:c + 1], beta_ap[:, c:c + 1],
                                            ALU.mult, ALU.add)
                return h

            st_pre = ln_stats_tile()
            for c in range(CT):
                shadow_update(c)
                ln_stats_mm(st_pre, c)

            for _rep in range(repeats):
                if _rep > 0:
                    nc.sync.dma_start(x_t[:], x0_d[:])
                    st_pre = ln_stats_tile()
                    for c in range(CT):
                        shadow_update(c)
                        ln_stats_mm(st_pre, c)
                for l in range(n_layers):
                    lnp_t = biasp.tile([128, 4 * CT], F32, tag="lnp")
                    nc.sync.dma_start(lnp_t[:], lnp_d[l])
                    bqk_t = biasp.tile([128, 2 * CT], F32, tag="bqk")
                    nc.sync.dma_start(bqk_t[:], bqk_d[l])
                    vbias_t = biasp.tile([128, C], BF16, tag="vbias")
                    nc.sync.dma_start(vbias_t[:], vbias_d[l])
                    bproj_t = biasp.tile([128, CT], F32, tag="bproj")
                    nc.sync.dma_start(bproj_t[:], bproj_d[l])
                    bff1_t = biasp.tile([128, FT], F32, tag="bff1")
                    nc.sync.dma_start(bff1_t[:], bff1_d[l])
                    bff2_t = biasp.tile([128, CT], F32, tag="bff2")
                    nc.sync.dma_start(bff2_t[:], bff2_d[l])

                    # prefetch this layer's weight streams upfront (SP queue
                    # order: wq first — K needs it earliest)
                    wq = []
                    for k in range(CT):
                        wt = wqkp.tile([128, 2 * C], BF16, tag="wqk")
                        nc.sync.dma_start(wt[:], wqk_d[l, :, k * 2 * C:(k + 1) * 2 * C])
                        wq.append(wt)
                    wv = []
                    for k in range(CT):
                        wt = wvp.tile([128, C], BF16, tag="wv")
                        nc.sync.dma_start(wt[:], wv_d[l, :, k * C:(k + 1) * C])
                        wv.append(wt)
                    wp = []
                    for k in range(CT):
                        wt = wprojp.tile([128, C], BF16, tag="wproj")
                        nc.sync.dma_start(wt[:], wproj_d[l, :, k * C:(k + 1) * C])
                        wp.append(wt)
                    w1 = []
                    for k in range(CT):
                        wt = wff1p.tile([128, DFF], BF16, tag="wff1")
                        nc.sync.dma_start(wt[:], wff1_d[l, :, k * DFF:(k + 1) * DFF])
                        w1.append(wt)
                    w2gs = {}
                    for gi in range(2):
                        w2g = wff2p.tile([128, 6 * C], BF16, tag="wff2")
                        nc.sync.dma_start(
                            w2g[:], wff2_d[l, :, gi * 6 * C:(gi + 1) * 6 * C])
                        w2gs[gi] = w2g

                    # ---- LN1 (stats were accumulated during prev residual) ----
                    h = layernorm_fin(st_pre, lnp_t[:, 0:CT], lnp_t[:, CT:2 * CT], "h")

                    qk = actp.tile([128, 2 * CT * TOK], BF16, tag="qk")

                    def qk_mm(d0, d1):
                        """two d-tiles of qk paired into one [128,512] psum bank.
                        ONE accumulation group (single start/stop) — a second
                        start would mark the whole bank pending-zero."""
                        ps = pmm.tile([128, 512], F32, tag="mm")
                        for w, d in ((0, d0), (1, d1)):
                            for k in range(CT):
                                nc.tensor.matmul(
                                    ps[:, w * 256:(w + 1) * 256],
                                    lhsT=wq[k][:, d * 128:(d + 1) * 128],
                                    rhs=h[:, k * TOK:(k + 1) * TOK],
                                    start=(w == 0 and k == 0),
                                    stop=(w == 1 and k == CT - 1),
                                    skip_group_check=True)
                        for w, d in ((0, d0), (1, d1)):
                            nc.vector.tensor_scalar_add(
                                qk[:, d * TOK:(d + 1) * TOK],
                                ps[:, w * 256:(w + 1) * 256], bqk_t[:, d:d + 1])

                    # ---- K matmuls first (d-tiles CT..2CT of qk) ----
                    for dp in range(CT // 2):
                        qk_mm(CT + 2 * dp, CT + 2 * dp + 1)

                    # stage K for the AllGather ASAP (int8 wire, beta-scaled)
                    kv_in = dramp.tile([128, KVW], I8, tag="kvin")
                    kv_out = dramp.tile([4, 128, KVW], I8, tag="kvout")
                    kq_i8 = pp.tile([128, 2 * VW], I8, tag="i8", bufs=1)
                    round_clip_i8(kq_i8[:, 0:CT * TOK], qk[:, CT * TOK:2 * CT * TOK])
                    # scalar (ACT) HWDGE queue: don't stall SP weight prefetches
                    nc.scalar.dma_start(kv_in[:, 0:CT * TOK], kq_i8[:, 0:CT * TOK])

                    # ---- V matmuls, token-major (x stationary) ----
                    v_i8 = pp.tile([128, 2 * VW], I8, tag="i8", bufs=1)
                    for tb in range(2):
                        for half in range(2):
                            vps = pmm.tile([128, 512], F32, tag="mm")
                            for k in range(CT):
                                nc.tensor.matmul(
                                    vps[:, 0:384],
                                    lhsT=h[:, k * TOK + tb * 128: k * TOK + tb * 128 + 128],
                                    rhs=wv[k][:, half * 384:(half + 1) * 384],
                                    start=(k == 0), stop=(k == CT - 1))
                            # scatter 6 heads' 64-col chunks into 65-col slots
                            dst = vloc[:, tb * VW: (tb + 1) * VW].rearrange(
                                "p (h c) -> p h c", c=VH)[:, half * 6:(half + 1) * 6, 0:DH]
                            src = vps[:, 0:384].rearrange("p (h c) -> p h c", c=DH)
                            vb = vbias_t[:, half * 384:(half + 1) * 384].rearrange(
                                "p (h c) -> p h c", c=DH)
                            nc.vector.tensor_tensor(dst, src, vb, ALU.add)
                        # round+stage this token-block's V as soon as both
                        # halves landed (AG issues ~1us earlier)
                        vs = slice(tb * VW, (tb + 1) * VW)
                        round_clip_i8(v_i8[:, vs], vloc[:, vs])
                        nc.scalar.dma_start(kv_in[:, CT * TOK + tb * VW:
                                                  CT * TOK + (tb + 1) * VW],
                                            v_i8[:, vs])

                    # ---- AllGather (V already staged per block) ----
                    if mock_cc:
                        for j in range(4):
                            nc.sync.dma_start(kv_out[j], kv_in[:])
                    else:
                        nc.gpsimd.collective_compute(
                            "AllGather", ALU.bypass, replica_groups=REPLICA_GROUPS,
                            ins=[kv_in.opt()], outs=[kv_out.opt()])

                    # ---- Q matmuls (overlap the collective) ----
                    for dp in range(CT // 2):
                        qk_mm(2 * dp, 2 * dp + 1)

                    # ---- fetch gathered KV (int8 -> bf16 cast); K regions
                    # first — the score matmuls need only K ----
                    kvsb = []
                    for j in range(4):
                        kt = kvsbp.tile([128, KVW], BF16, tag="kvsb")
                        nc.gpsimd.dma_start(kt[:, 0:CT * TOK],
                                            kv_out[j][:, 0:CT * TOK])
                        kvsb.append(kt)
                    for j in range(4):
                        nc.gpsimd.dma_start(kvsb[j][:, CT * TOK:],
                                            kv_out[j][:, CT * TOK:])

                    def k_ap(ro, dt, n):
                        j, s = (n, 0) if n < 4 else (7 - n, 1)
                        base = dt * TOK + s * 128
                        return kvsb[j][ro:ro + 64, base:base + 128]

                    def v_ap(hd, n):
                        j, s = (n, 0) if n < 4 else (7 - n, 1)
                        base = CT * TOK + s * VW + hd * VH
                        return kvsb[j][:, base:base + VH]

                    # ---- attention, per head-pair; batched exp/masks ----
                    attn = actp.tile([128, CT * TOK], BF16, tag="attn")
                    for dt in range(CT):
                        hd0, hd1 = 2 * dt, 2 * dt + 1
                        q0 = qk[0:64, dt * TOK:(dt + 1) * TOK]
                        q1 = qk[64:128, dt * TOK:(dt + 1) * TOK]
                        q0B = qk[0:64, dt * TOK + 128:(dt + 1) * TOK]
                        q1B = qk[64:128, dt * TOK + 128:(dt + 1) * TOK]
                        # scores strip: n<4 at n*256 (qA|qB), n>=4 at 1024+(n-4)*128
                        # one accumulation group per 512-col bank:
                        # bank0={0,1} bank1={2,3} bank2={4,5,6,7}
                        s0 = pscore.tile([128, 1536], F32, tag="sc")
                        s1 = pscore.tile([128, 1536], F32, tag="sc")
                        for n in range(4):
                            o = slice(n * 256, (n + 1) * 256)
                            st, sp = (n % 2 == 0), (n % 2 == 1)
                            nc.tensor.matmul(s0[:, o], lhsT=k_ap(0, dt, n),
                                             rhs=q0, start=st, stop=sp,
                                             skip_group_check=True)
                            nc.tensor.matmul(s1[:, o], lhsT=k_ap(64, dt, n),
                                             rhs=q1, start=st, stop=sp,
                                             skip_group_check=True)
                        for n in range(4, 8):
                            o = slice(1024 + (n - 4) * 128, 1024 + (n - 3) * 128)
                            st, sp = (n == 4), (n == 7)
                            nc.tensor.matmul(s0[:, o], lhsT=k_ap(0, dt, n),
                                             rhs=q0B, start=st, stop=sp,
                                             skip_group_check=True)
                            nc.tensor.matmul(s1[:, o], lhsT=k_ap(64, dt, n),
                                             rhs=q1B, start=st, stop=sp,
                                             skip_group_check=True)
                        # one exp per strip
                        p0 = pp.tile([128, 1536], BF16, tag="p", bufs=3)
                        p1 = pp.tile([128, 1536], BF16, tag="p", bufs=3)
                        nc.scalar.activation(p0[:], s0[:], AF.Exp)
                        nc.scalar.activation(p1[:], s1[:], AF.Exp)
                        # masks: qA cols of n<4 (per block), then n>=4 as one op
                        for p in (p0, p1):
                            for n in range(4):
                                nc.vector.tensor_tensor(
                                    p[:, n * 256:n * 256 + 128],
                                    p[:, n * 256:n * 256 + 128],
                                    maskA_t[:, n * 128:(n + 1) * 128], ALU.mult)
                            nc.vector.tensor_tensor(p[:, 1024:1536], p[:, 1024:1536],
                                                    maskB_t[:], ALU.mult)
                        # AV accumulation: po packed [hd0: 0:256 | hd1: 256:512] —
                        # ONE group (single start on first MM, stop on last).
                        po = pmm.tile([128, 512], F32, tag="mm")
                        for n in range(NB):
                            if n < 4:
                                r0 = p0[:, n * 256:(n + 1) * 256]
                                r1 = p1[:, n * 256:(n + 1) * 256]
                                o0, o1 = slice(0, 256), slice(256, 512)
                            else:
                                r0 = p0[:, 1024 + (n - 4) * 128:1024 + (n - 3) * 128]
                                r1 = p1[:, 1024 + (n - 4) * 128:1024 + (n - 3) * 128]
                                o0, o1 = slice(128, 256), slice(384, 512)
                            nc.tensor.matmul(po[0:VH, o0], lhsT=v_ap(hd0, n),
                                             rhs=r0, start=(n == 0), stop=False,
                                             skip_group_check=True)
                            nc.tensor.matmul(po[0:VH, o1], lhsT=v_ap(hd1, n),
                                             rhs=r1, start=False, stop=(n == NB - 1),
                                             skip_group_check=True)
                        # normalize: reciprocal of the ones-row, broadcast, multiply
                        rc = statsp.tile([1, 512], BF16, tag="rc")
                        with nc.allow_low_precision(reason="softmax recip to bf16"):
                            nc.vector.reciprocal(rc[:], po[DH:VH, :])
                        rs = statsp.tile([64, 512], BF16, tag="rs")
                        nc.gpsimd.partition_broadcast(rs[:], rc[:], channels=64)
                        nc.vector.tensor_tensor(attn[0:64, dt * TOK:(dt + 1) * TOK],
                                                po[0:DH, 0:256], rs[:, 0:256], ALU.mult)
                        nc.vector.tensor_tensor(attn[64:128, dt * TOK:(dt + 1) * TOK],
                                                po[0:DH, 256:512], rs[:, 256:512],
                                                ALU.mult)

                    # ---- proj + residual (+ shadow refresh + LN2 stats) ----
                    st_ln2 = ln_stats_tile()
                    for dp in range(CT // 2):
                        ps = pmm.tile([128, 512], F32, tag="mm")
                        for w, d in ((0, 2 * dp), (1, 2 * dp + 1)):
                            for k in range(CT):
                                nc.tensor.matmul(
                                    ps[:, w * 256:(w + 1) * 256],
                                    lhsT=wp[k][:, d * 128:(d + 1) * 128],
                                    rhs=attn[:, k * TOK:(k + 1) * TOK],
                                    start=(w == 0 and k == 0),
                                    stop=(w == 1 and k == CT - 1),
                                    skip_group_check=True)
                        for w, d in ((0, 2 * dp), (1, 2 * dp + 1)):
                            nc.vector.scalar_tensor_tensor(
                                out=x_t[:, d * TOK:(d + 1) * TOK],
                                in0=ps[:, w * 256:(w + 1) * 256],
                                scalar=bproj_t[:, d:d + 1],
                                in1=x_t[:, d * TOK:(d + 1) * TOK],
                                op0=ALU.add, op1=ALU.add)
                            shadow_update(d)
                            ln_stats_mm(st_ln2, d)

                    # ---- LN2 + MLP ----
                    h2 = layernorm_fin(st_ln2, lnp_t[:, 2 * CT:3 * CT],
                                       lnp_t[:, 3 * CT:4 * CT], "h")
                    g = actp.tile([128, FT * TOK], BF16, tag="g")
                    for dp in range(FT // 2):
                        ps = pmm.tile([128, 512], F32, tag="mm")
                        for w, d in ((0, 2 * dp), (1, 2 * dp + 1)):
                            for k in range(CT):
                                nc.tensor.matmul(
                                    ps[:, w * 256:(w + 1) * 256],
                                    lhsT=w1[k][:, d * 128:(d + 1) * 128],
                                    rhs=h2[:, k * TOK:(k + 1) * TOK],
                                    start=(w == 0 and k == 0),
                                    stop=(w == 1 and k == CT - 1),
                                    skip_group_check=True)
                        for w, d in ((0, 2 * dp), (1, 2 * dp + 1)):
                            nc.scalar.activation(g[:, d * TOK:(d + 1) * TOK],
                                                 ps[:, w * 256:(w + 1) * 256],
                                                 AF.Gelu, bias=bff1_t[:, d:d + 1])

                    # dummy 1-elem Ln right after the gelus: hoists the
                    # natural_log_exp table reload into the ff2 window instead
                    # of the next layer's LN1 critical chain
                    dmy = statsp.tile([1, 1], F32, tag="dmy", bufs=1)
                    nc.scalar.activation(dmy[:], eps_t[0:1, 0:1], AF.Ln)
                    # ff2 k-outer: one [128,1536] strip holds all 6 d-chains
                    # (one group per bank); w2 streamed in 4 groups of 6
    				# k-tiles (1.15MB DMAs, bufs=2) — frees ~19KB SBUF vs
                    # keeping all 24 k-tiles resident.
                    f2 = pscore.tile([128, 1536], F32, tag="sc")
                    for k in range(FT):
                        gi = k // 6
                        if k % 6 == 0 and gi + 2 < 4 and gi + 2 not in w2gs:
                            # prefetch one group ahead: group gi+2's buf frees
                            # as soon as group gi's last MM retires
                            w2g = wff2p.tile([128, 6 * C], BF16, tag="wff2")
                            nc.sync.dma_start(
                                w2g[:],
                                wff2_d[l, :, (gi + 2) * 6 * C:(gi + 3) * 6 * C])
                            w2gs[gi + 2] = w2g
                        w2g = w2gs[gi]
                        for d in range(CT):
                            nc.tensor.matmul(
                                f2[:, d * 256:(d + 1) * 256],
                                lhsT=w2g[:, (k % 6) * C + d * 128:
                                         (k % 6) * C + (d + 1) * 128],
                                rhs=g[:, k * TOK:(k + 1) * TOK],
                                start=(k == 0 and d % 2 == 0),
                                stop=(k == FT - 1 and d % 2 == 1),
                                skip_group_check=True)
                    st_pre = ln_stats_tile()
                    for d in range(CT):
                        nc.vector.scalar_tensor_tensor(
                            out=x_t[:, d * TOK:(d + 1) * TOK],
                            in0=f2[:, d * 256:(d + 1) * 256],
                            scalar=bff2_t[:, d:d + 1],
                            in1=x_t[:, d * TOK:(d + 1) * TOK],
                            op0=ALU.add, op1=ALU.add)
                        shadow_update(d)
                        ln_stats_mm(st_pre, d)

                # ---- final LN + token-sharded head (no collective!) ----
                # own 2 blocks x full vocab, streaming 1000-vocab chunks.
                xf = layernorm_fin(st_pre, lnf_t[:, 0:CT], lnf_t[:, CT:2 * CT], "h")
                for v in range(NH2):
                    wh = wff1p.tile([128, CT * HV2], BF16, tag="whd", bufs=2)
                    nc.sync.dma_start(wh[:], whead_d[v])
                    for s in range(2):
                        for hf in range(2):
                            ps = pmm.tile([128, 512], F32, tag="mm")
                            for k in range(CT):
                                nc.tensor.matmul(
                                    ps[:, 0:HV_CHUNK],
                                    lhsT=xf[:, k * TOK + s * 128:
                                            k * TOK + s * 128 + 128],
                                    rhs=wh[:, k * HV2 + hf * HV_CHUNK:
                                            k * HV2 + (hf + 1) * HV_CHUNK],
                                    start=(k == 0), stop=(k == CT - 1))
                            ot = statsp.tile([128, HV_CHUNK], BF16, tag="lout", bufs=2)
                            nc.vector.tensor_copy(ot[:], ps[:, 0:HV_CHUNK])
                            nc.scalar.dma_start(
                                logits_d[s * 128:(s + 1) * 128,
                                         v * HV2 + hf * HV_CHUNK:
                                         v * HV2 + (hf + 1) * HV_CHUNK],
                                ot[:])

    nc.compile()
    return nc


_NC_CACHE = {}


def _get_nc(n_layers=L, repeats=1):
    key = (n_layers, repeats)
    if key not in _NC_CACHE:
        _NC_CACHE[key] = build_kernel(n_layers, repeats)
    return _NC_CACHE[key]


def _to_bf16(a):
    return np.asarray(a, dtype=ml_dtypes.bfloat16)


def _colblock(w):
    """[K, D] -> [128, (K//128)*D] col-block layout: col block k = w[128k:128k+128, :]."""
    K, D = w.shape
    return np.concatenate([w[k * 128:(k + 1) * 128, :] for k in range(K // 128)], axis=1)


def _perchan(v):
    """[768] (or [n*128]) -> [128, n] per-partition layout."""
    n = v.shape[0] // 128
    return v.reshape(n, 128).T.copy()


def prepare_inputs(idx, tok_emb, pos_emb, ln1_g, ln1_b, qkv_w, qkv_b, proj_w, proj_b,
                   ln2_g, ln2_b, ff1_w, ff1_b, ff2_w, ff2_b, lnf_g, lnf_b, head_w):
    """Build the 8 per-core input maps (host-side shard + transpose + cast)."""
    idx = np.asarray(idx)
    emb = tok_emb[idx] + pos_emb[None, :, :]          # [2, 1024, 768] f32
    tri = np.triu(np.ones((128, 128), np.float32))    # mask[k, q] = k <= q

    # weights (shared across cores except head)
    wqk = np.empty((L, 128, CT * 2 * C), ml_dtypes.bfloat16)
    wv = np.empty((L, 128, CT * C), ml_dtypes.bfloat16)
    wproj = np.empty((L, 128, CT * C), ml_dtypes.bfloat16)
    wff1 = np.empty((L, 128, CT * DFF), ml_dtypes.bfloat16)
    wff2 = np.empty((L, 128, FT * C), ml_dtypes.bfloat16)
    bqk = np.empty((L, 128, 2 * CT), np.float32)
    vbias = np.empty((L, 128, C), ml_dtypes.bfloat16)
    bproj = np.empty((L, 128, CT), np.float32)
    bff1 = np.empty((L, 128, FT), np.float32)
    bff2 = np.empty((L, 128, CT), np.float32)
    lnp = np.empty((L, 128, 4 * CT), np.float32)
    for l in range(L):
        wqk_l = qkv_w[l][:2 * C].T.astype(np.float32).copy()   # [768, 1536]
        wqk_l[:, :C] *= 1.0 / (np.sqrt(DH) * BETA)             # fold q scaling + 1/beta
        wqk_l[:, C:] *= BETA                                   # K rides beta on the wire
        bq = qkv_b[l][:2 * C].astype(np.float32).copy()
        bq[:C] *= 1.0 / (np.sqrt(DH) * BETA)
        bq[C:] *= BETA
        wqk[l] = _to_bf16(_colblock(wqk_l))
        # V weight in [c, d] layout (x-stationary matmul); alpha on the wire
        wv[l] = _to_bf16(_colblock(qkv_w[l][2 * C:].T.astype(np.float32) * ALPHA))
        wproj[l] = _to_bf16(_colblock(proj_w[l].T.astype(np.float32)))
        wff1[l] = _to_bf16(_colblock(ff1_w[l].T.astype(np.float32)))
        wff2[l] = _to_bf16(_colblock(ff2_w[l].T.astype(np.float32)))
        bqk[l] = _perchan(bq)
        vbias[l] = _to_bf16(np.broadcast_to(
            qkv_b[l][2 * C:].astype(np.float32)[None, :] * ALPHA, (128, C)))
        bproj[l] = _perchan(proj_b[l].astype(np.float32))
        bff1[l] = _perchan(ff1_b[l].astype(np.float32))
        bff2[l] = _perchan(ff2_b[l].astype(np.float32))
        lnp[l] = np.concatenate(
            [_perchan(a[l].astype(np.float32)) for a in (ln1_g, ln1_b, ln2_g, ln2_b)],
            axis=1)
    lnf = np.concatenate([_perchan(lnf_g.astype(np.float32)),
                          _perchan(lnf_b.astype(np.float32))], axis=1)
    ones = np.ones((128, 128), ml_dtypes.bfloat16)

    # full-vocab head, shared by all cores (token-sharded head):
    # [768, 32000] -> colblock [128, 6*32000] -> chunks [32, 128, 6*1000]
    whT = _to_bf16(_colblock(head_w.T.astype(np.float32)))  # [128, 6*32000]
    whead = np.ascontiguousarray(
        whT.reshape(128, CT, NH2, HV2).transpose(2, 0, 1, 3).reshape(
            NH2, 128, CT * HV2))

    in_maps = []
    for core in range(8):
        grp, j = core // 4, core % 4
        pA, pB = j, 7 - j
        xT = emb[grp].T.astype(np.float32)            # [768, 1024]
        x0 = np.empty((128, CT * TOK), np.float32)
        for c in range(CT):
            rows = xT[c * 128:(c + 1) * 128]
            x0[:, c * TOK:c * TOK + 128] = rows[:, pA * 128:(pA + 1) * 128]
            x0[:, c * TOK + 128:(c + 1) * TOK] = rows[:, pB * 128:(pB + 1) * 128]
        mA = np.empty((128, 4 * 128), np.float32)
        mB = np.empty((128, 4 * 128), np.float32)
        for n in range(4):
            mA[:, n * 128:(n + 1) * 128] = (
                1.0 if n < pA else tri if n == pA else 0.0)
            nb = n + 4
            mB[:, n * 128:(n + 1) * 128] = (
                1.0 if nb < pB else tri if nb == pB else 0.0)
        in_maps.append({
            "x0": x0, "wqk": wqk, "wv": wv, "wproj": wproj, "wff1": wff1,
            "wff2": wff2, "whead": whead, "bqk": bqk, "vbias": vbias,
            "bproj": bproj, "bff1": bff1, "bff2": bff2, "lnp": lnp, "lnf": lnf,
            "maskA": _to_bf16(mA), "maskB": _to_bf16(mB),
            "ones": ones,
        })
    return in_maps


def run(in_maps, n_layers=L, trace=False, **kw):
    nc = _get_nc(n_layers)
    return run_bass_kernel_spmd(nc, in_maps, list(range(8)), trace=trace, **kw)


def kernel(**inputs):
    in_maps = prepare_inputs(**inputs)
    res = run(in_maps)
    out = np.empty((2, T, V), np.float32)
    for core in range(8):
        grp, j = core // 4, core % 4
        lg = np.asarray(res.results[core]["logits"], dtype=np.float32)
        out[grp, j * 128:(j + 1) * 128, :] = lg[0:128]
        out[grp, (7 - j) * 128:(8 - j) * 128, :] = lg[128:256]
    return out


if __name__ == "__main__":
    pass
